# revision 1
# baseline (speedup 1.0000x reference)
"""nn_Decoder LSTM kernel for 8 TRN2 NeuronCores (self-contained).

Sharding: model-parallel over the hidden/gate dim. Each core owns a 128-wide
slice of H=1024 (= 512 of the 4096 gate rows, column order [g|i|f|o]).
Host-side, the input projection is folded into the embedding table
(emb_proj = emb @ W_ih.T + b_ih + b_hh, restricted to the used vocab), so
the device-side input projection is a pure indirect-DMA row gather.

Per step, per core:
  gates[64,512](PSUM,f32) = I64.T @ xp_t  +  sum_r hT[:,r,:].T @ W_rec[:,r,:]
  LSTM cell elementwise on [64,128] slices (ScalarE tanh/sigmoid, VectorE)
  h slice -> PE transpose -> bf16 [128,64] -> AllGather across the 8 cores
c stays core-local; outputs stream to DRAM each step.
"""

from contextlib import ExitStack

import numpy as np

import concourse.bass as bass
import concourse.mybir as mybir
from concourse.tile import TileContext
from concourse.masks import make_identity
from concourse.bass_utils import run_bass_kernel_spmd

import bass_rust
import concourse.tile as tile_mod
from concourse.vector_clock import ScopedClock

B = 64
T = 256
H = 1024
NC = 8
HS = H // NC
G = 4 * HS
KCH = H // 128

F32 = mybir.dt.float32
BF16 = mybir.dt.bfloat16
I32 = mybir.dt.int32


def _patched_drain_and_barrier(self, tick_clock, wait_clock):
    # This walrus build rejects >1 sync-wait per instruction; spread the
    # final drain's waits across single-wait nops.
    carrier = self.nc.sync.nop()
    wait_clock.add_sem_waits(carrier.ins, ScopedClock({None: tick_clock.global_clock}))
    waits = list(carrier.ins.sync_info.on_wait)
    carrier.ins.sync_info = bass_rust.SyncInfo(on_wait=waits[:1], on_update=[])
    for w in waits[1:]:
        n = self.nc.sync.nop()
        n.ins.sync_info = bass_rust.SyncInfo(on_wait=[w], on_update=[])
    self.nc.sync.drain()
    self.nc.all_engine_barrier()
    popped = self.nc._tile_sem_poison_stack.pop()
    assert popped is self._sem_poison
    self.nc.clear_and_free_semaphores(list(self.sems.allocated().values()))
    self.nc.all_engine_barrier()


tile_mod.TileContext._drain_and_barrier = _patched_drain_and_barrier


def _split_multi_waits(nc):
    n_split = 0
    for fn in nc.m.functions:
        for blk in fn.blocks:
            out = []
            for inst in blk.instructions:
                si = inst.sync_info
                if si is not None and len(si.on_wait) > 1:
                    waits = list(si.on_wait)
                    for w in waits[:-1]:
                        n_split += 1
                        out.append(
                            mybir.InstNoOp(
                                name=f"{inst.name}-w{n_split}",
                                sync_info=mybir.SyncInfo(on_wait=[w], on_update=[]),
                                bass_nofuse=True,
                                engine=inst.engine,
                            )
                        )
                    inst.sync_info = mybir.SyncInfo(
                        on_wait=waits[-1:], on_update=list(si.on_update)
                    )
                out.append(inst)
            blk.instructions = out
    return n_split


def _build_nc(vocab: int):
    nc = bass.Bass("TRN2", target_bir_lowering=False, debug=False, num_devices=NC)

    w_rec = nc.declare_dram_parameter("w_rec", [128, KCH, G], BF16, isOutput=False)
    emb_proj = nc.declare_dram_parameter("emb_proj", [vocab, G], BF16, isOutput=False)
    tgt_idx = nc.declare_dram_parameter("tgt_idx", [B, T], I32, isOutput=False)
    h0T = nc.declare_dram_parameter("h0T", [128, KCH, B], BF16, isOutput=False)
    c0_s = nc.declare_dram_parameter("c0_s", [B, HS], F32, isOutput=False)

    out_h = nc.declare_dram_parameter("out_h", [T, B, HS], F32, isOutput=True)
    c_out = nc.declare_dram_parameter("c_out", [B, HS], F32, isOutput=True)

    rg = [list(range(NC))]

    with TileContext(nc) as tc, ExitStack() as ctx:
        consts = ctx.enter_context(tc.tile_pool(name="consts", bufs=1))
        state = ctx.enter_context(tc.tile_pool(name="state", bufs=1))
        xpp = ctx.enter_context(tc.tile_pool(name="xp", bufs=6))
        htp = ctx.enter_context(tc.tile_pool(name="ht", bufs=2))
        work = ctx.enter_context(tc.tile_pool(name="work", bufs=3))
        hsl_p = ctx.enter_context(tc.tile_pool(name="hsl", bufs=2))
        gpsum = ctx.enter_context(tc.tile_pool(name="gpsum", bufs=2, space="PSUM"))
        tpsum = ctx.enter_context(tc.tile_pool(name="tpsum", bufs=2, space="PSUM"))
        dram = ctx.enter_context(tc.tile_pool(name="dram", bufs=2, space="DRAM"))

        w_t = consts.tile([128, KCH, G], BF16)
        nc.sync.dma_start(out=w_t[:], in_=w_rec[:])
        tgt_t = consts.tile([B, T], I32)
        nc.sync.dma_start(out=tgt_t[:], in_=tgt_idx[:])
        id_f = consts.tile([64, 64], F32)
        make_identity(nc, id_f[:])
        id_b = consts.tile([64, 64], BF16)
        nc.vector.tensor_copy(id_b[:], id_f[:])

        c_t = state.tile([B, HS], F32)
        nc.sync.dma_start(out=c_t[:], in_=c0_s[:])

        hT = htp.tile([128, KCH, B], BF16)
        nc.sync.dma_start(out=hT[:], in_=h0T[:])

        for t in range(T):
            xp = xpp.tile([B, G], BF16)
            nc.gpsimd.indirect_dma_start(
                out=xp[:],
                out_offset=None,
                in_=emb_proj[:],
                in_offset=bass.IndirectOffsetOnAxis(ap=tgt_t[:, t : t + 1], axis=0),
            )

            gp = gpsum.tile([B, G], F32)
            nc.tensor.matmul(gp[:], id_b[:], xp[:], start=True, stop=False)
            for r in range(KCH):
                nc.tensor.matmul(
                    gp[:], hT[:, r, :], w_t[:, r, :],
                    start=False, stop=(r == KCH - 1),
                )

            # PSUM gate layout: [g | i | f | o], 128 cols each
            sg = work.tile([B, 3 * HS], F32)
            nc.scalar.activation(sg[:], gp[:, HS:G], mybir.ActivationFunctionType.Sigmoid)
            tg = work.tile([B, HS], F32)
            nc.scalar.activation(tg[:], gp[:, 0:HS], mybir.ActivationFunctionType.Tanh)

            t2 = work.tile([B, HS], F32)
            nc.vector.tensor_mul(t2[:], sg[:, HS : 2 * HS], c_t[:])  # sig(f)*c
            t1 = work.tile([B, HS], F32)
            nc.vector.tensor_mul(t1[:], sg[:, 0:HS], tg[:])          # sig(i)*tanh(g)
            nc.vector.tensor_add(c_t[:], t1[:], t2[:])
            tc_ = work.tile([B, HS], F32)
            nc.scalar.activation(tc_[:], c_t[:], mybir.ActivationFunctionType.Tanh)
            hnew = work.tile([B, HS], F32)
            nc.vector.tensor_mul(hnew[:], sg[:, 2 * HS : 3 * HS], tc_[:])

            nc.sync.dma_start(out=out_h[t], in_=hnew[:])

            if t < T - 1:
                tp = tpsum.tile([HS, B], F32)
                nc.tensor.transpose(tp[:], hnew[:], id_f[:])
                hsl = hsl_p.tile([HS, B], BF16)
                nc.vector.tensor_copy(hsl[:], tp[:])

                inb = dram.tile([HS, B], BF16)
                nc.sync.dma_start(out=inb[:], in_=hsl[:])
                outb = dram.tile([NC * HS, B], BF16)
                nc.gpsimd.collective_compute(
                    "AllGather",
                    mybir.AluOpType.bypass,
                    replica_groups=rg,
                    ins=[inb.opt()],
                    outs=[outb.opt()],
                )
                hT = htp.tile([128, KCH, B], BF16)
                nc.sync.dma_start(
                    out=hT[:], in_=outb[:].rearrange("(r p) n -> p r n", p=128)
                )

        nc.sync.dma_start(out=c_out[:], in_=c_t[:])

    _split_multi_waits(nc)
    return nc


def _host_emb_proj(emb, W_ih, b_ih, b_hh, tgt):
    used, tgt_c = np.unique(tgt, return_inverse=True)
    tgt_c = tgt_c.reshape(tgt.shape)
    bias = (b_ih + b_hh).astype(np.float32)
    ep = emb[used].astype(np.float32) @ W_ih.T.astype(np.float32) + bias
    return ep.astype(np.float32), tgt_c.astype(np.int32)


def _prep_core_inputs(tgt_c, h0, c0, emb_proj_full, W_hh):
    from ml_dtypes import bfloat16

    def slice_rows(M, k):  # rows for core k in [g,i,f,o] order
        return np.concatenate(
            [
                M[2 * H + k * HS : 2 * H + (k + 1) * HS],
                M[0 * H + k * HS : 0 * H + (k + 1) * HS],
                M[1 * H + k * HS : 1 * H + (k + 1) * HS],
                M[3 * H + k * HS : 3 * H + (k + 1) * HS],
            ],
            axis=0,
        )

    h0T_np = (
        np.ascontiguousarray(h0[0].astype(np.float32).T)
        .reshape(KCH, 128, B)
        .transpose(1, 0, 2)
        .astype(bfloat16)
    )
    in_maps = []
    for k in range(NC):
        w_sl = slice_rows(W_hh.astype(np.float32), k)        # [G, H]
        w_rec = (
            np.ascontiguousarray(w_sl.T)
            .reshape(KCH, 128, G)
            .transpose(1, 0, 2)
            .astype(bfloat16)
        )
        ep_sl = np.ascontiguousarray(slice_rows(emb_proj_full.T, k).T)
        in_maps.append(
            {
                "w_rec": np.ascontiguousarray(w_rec),
                "emb_proj": ep_sl.astype(bfloat16),
                "tgt_idx": np.ascontiguousarray(tgt_c),
                "h0T": np.ascontiguousarray(h0T_np),
                "c0_s": np.ascontiguousarray(
                    c0[0].astype(np.float32)[:, k * HS : (k + 1) * HS]
                ),
            }
        )
    return in_maps


_CACHE = {}


def _run(inputs, trace=False):
    tgt = np.asarray(inputs["tgt"])
    h0 = np.asarray(inputs["h0"])
    c0 = np.asarray(inputs["c0"])
    emb = np.asarray(inputs["emb"])
    W_ih = np.asarray(inputs["W_ih"])
    W_hh = np.asarray(inputs["W_hh"])
    b_ih = np.asarray(inputs["b_ih"])
    b_hh = np.asarray(inputs["b_hh"])

    ep, tgt_c = _host_emb_proj(emb, W_ih, b_ih, b_hh, tgt)
    in_maps = _prep_core_inputs(tgt_c, h0, c0, ep, W_hh)

    key = ("nc", ep.shape[0])
    if key not in _CACHE:
        _CACHE[key] = _build_nc(ep.shape[0])
    nc = _CACHE[key]

    res = run_bass_kernel_spmd(nc, in_maps, core_ids=list(range(NC)), trace=trace)

    outs = np.concatenate([res.results[k]["out_h"] for k in range(NC)], axis=2)
    outputs = np.ascontiguousarray(outs.transpose(1, 0, 2)).astype(np.float32)
    c_fin = np.concatenate([res.results[k]["c_out"] for k in range(NC)], axis=1)[None]
    h_fin = np.ascontiguousarray(outputs[:, T - 1, :])[None]
    return (outputs, (h_fin.astype(np.float32), c_fin.astype(np.float32))), res


def kernel(**inputs):
    out, _ = _run(inputs, trace=False)
    return out


# revision 2
# speedup vs baseline: 1.0124x; 1.0124x over previous
"""nn_Decoder LSTM kernel for 8 TRN2 NeuronCores (self-contained).

Sharding: model-parallel over the hidden/gate dim. Each core owns a 128-wide
slice of H=1024 (= 512 of the 4096 gate rows, column order [g|i|f|o]).
Host-side, the input projection is folded into the embedding table
(emb_proj = emb @ W_ih.T + b_ih + b_hh, restricted to the used vocab), so
the device-side input projection is a pure indirect-DMA row gather.

Per step, per core:
  gates[64,512](PSUM,f32) = I64.T @ xp_t  +  sum_r hT[:,r,:].T @ W_rec[:,r,:]
  LSTM cell elementwise on [64,128] slices (ScalarE tanh/sigmoid, VectorE)
  h slice -> PE transpose -> bf16 [128,64] -> AllGather across the 8 cores
c stays core-local; outputs stream to DRAM each step.
"""

from contextlib import ExitStack

import numpy as np

import concourse.bass as bass
import concourse.mybir as mybir
from concourse.tile import TileContext
from concourse.masks import make_identity
from concourse.bass_utils import run_bass_kernel_spmd

import bass_rust
import concourse.tile as tile_mod
from concourse.vector_clock import ScopedClock

B = 64
T = 256
H = 1024
NC = 8
HS = H // NC
G = 4 * HS
KCH = H // 128

F32 = mybir.dt.float32
BF16 = mybir.dt.bfloat16
I32 = mybir.dt.int32


def _patched_drain_and_barrier(self, tick_clock, wait_clock):
    # This walrus build rejects >1 sync-wait per instruction; spread the
    # final drain's waits across single-wait nops.
    carrier = self.nc.sync.nop()
    wait_clock.add_sem_waits(carrier.ins, ScopedClock({None: tick_clock.global_clock}))
    waits = list(carrier.ins.sync_info.on_wait)
    carrier.ins.sync_info = bass_rust.SyncInfo(on_wait=waits[:1], on_update=[])
    for w in waits[1:]:
        n = self.nc.sync.nop()
        n.ins.sync_info = bass_rust.SyncInfo(on_wait=[w], on_update=[])
    self.nc.sync.drain()
    self.nc.all_engine_barrier()
    popped = self.nc._tile_sem_poison_stack.pop()
    assert popped is self._sem_poison
    self.nc.clear_and_free_semaphores(list(self.sems.allocated().values()))
    self.nc.all_engine_barrier()


tile_mod.TileContext._drain_and_barrier = _patched_drain_and_barrier


def _split_multi_waits(nc):
    n_split = 0
    for fn in nc.m.functions:
        for blk in fn.blocks:
            out = []
            for inst in blk.instructions:
                si = inst.sync_info
                if si is not None and len(si.on_wait) > 1:
                    waits = list(si.on_wait)
                    for w in waits[:-1]:
                        n_split += 1
                        out.append(
                            mybir.InstNoOp(
                                name=f"{inst.name}-w{n_split}",
                                sync_info=mybir.SyncInfo(on_wait=[w], on_update=[]),
                                bass_nofuse=True,
                                engine=inst.engine,
                            )
                        )
                    inst.sync_info = mybir.SyncInfo(
                        on_wait=waits[-1:], on_update=list(si.on_update)
                    )
                out.append(inst)
            blk.instructions = out
    return n_split


def _build_nc(vocab: int):
    nc = bass.Bass("TRN2", target_bir_lowering=False, debug=False, num_devices=NC)

    w_rec = nc.declare_dram_parameter("w_rec", [128, KCH, G], BF16, isOutput=False)
    emb_proj = nc.declare_dram_parameter("emb_proj", [vocab, G], BF16, isOutput=False)
    tgt_idx = nc.declare_dram_parameter("tgt_idx", [B, T], I32, isOutput=False)
    h0T = nc.declare_dram_parameter("h0T", [128, KCH, B], BF16, isOutput=False)
    c0_s = nc.declare_dram_parameter("c0_s", [B, HS], F32, isOutput=False)

    out_h = nc.declare_dram_parameter("out_h", [T, B, HS], F32, isOutput=True)
    c_out = nc.declare_dram_parameter("c_out", [B, HS], F32, isOutput=True)

    rg = [list(range(NC))]

    with TileContext(nc) as tc, ExitStack() as ctx:
        consts = ctx.enter_context(tc.tile_pool(name="consts", bufs=1))
        state = ctx.enter_context(tc.tile_pool(name="state", bufs=1))
        xpp = ctx.enter_context(tc.tile_pool(name="xp", bufs=6))
        htp = ctx.enter_context(tc.tile_pool(name="ht", bufs=2))
        work = ctx.enter_context(tc.tile_pool(name="work", bufs=4))
        hsl_p = ctx.enter_context(tc.tile_pool(name="hsl", bufs=2))
        gpsum = ctx.enter_context(tc.tile_pool(name="gpsum", bufs=3, space="PSUM"))
        tpsum = ctx.enter_context(tc.tile_pool(name="tpsum", bufs=3, space="PSUM"))
        dram = ctx.enter_context(tc.tile_pool(name="dram", bufs=2, space="DRAM"))

        w_t = consts.tile([128, KCH, G], BF16)
        nc.sync.dma_start(out=w_t[:], in_=w_rec[:])
        tgt_t = consts.tile([B, T], I32)
        nc.sync.dma_start(out=tgt_t[:], in_=tgt_idx[:])
        id_f = consts.tile([64, 64], F32)
        make_identity(nc, id_f[:])
        id_b = consts.tile([64, 64], BF16)
        nc.vector.tensor_copy(id_b[:], id_f[:])

        c_t = state.tile([B, HS], F32)
        nc.sync.dma_start(out=c_t[:], in_=c0_s[:])

        hT = htp.tile([128, KCH, B], BF16)
        nc.sync.dma_start(out=hT[:], in_=h0T[:])

        for t in range(T):
            xp = xpp.tile([B, G], BF16)
            nc.gpsimd.indirect_dma_start(
                out=xp[:],
                out_offset=None,
                in_=emb_proj[:],
                in_offset=bass.IndirectOffsetOnAxis(ap=tgt_t[:, t : t + 1], axis=0),
            )

            gp = gpsum.tile([B, G], F32)
            nc.tensor.matmul(gp[:], id_b[:], xp[:], start=True, stop=False)
            for r in range(KCH):
                nc.tensor.matmul(
                    gp[:], hT[:, r, :], w_t[:, r, :],
                    start=False, stop=(r == KCH - 1),
                )

            # PSUM gate layout: [g | i | f | o], 128 cols each
            sg = work.tile([B, 3 * HS], F32)
            nc.scalar.activation(sg[:], gp[:, HS:G], mybir.ActivationFunctionType.Sigmoid)
            tg = work.tile([B, HS], F32)
            nc.scalar.activation(tg[:], gp[:, 0:HS], mybir.ActivationFunctionType.Tanh)

            t2 = work.tile([B, HS], F32)
            nc.vector.tensor_mul(t2[:], sg[:, HS : 2 * HS], c_t[:])  # sig(f)*c
            t1 = work.tile([B, HS], F32)
            nc.vector.tensor_mul(t1[:], sg[:, 0:HS], tg[:])          # sig(i)*tanh(g)
            nc.vector.tensor_add(c_t[:], t1[:], t2[:])
            tc_ = work.tile([B, HS], F32)
            nc.scalar.activation(tc_[:], c_t[:], mybir.ActivationFunctionType.Tanh)
            hnew = work.tile([B, HS], F32)
            nc.vector.tensor_mul(hnew[:], sg[:, 2 * HS : 3 * HS], tc_[:])

            if t < T - 1:
                tp = tpsum.tile([HS, B], F32)
                nc.tensor.transpose(tp[:], hnew[:], id_f[:])
                hsl = hsl_p.tile([HS, B], BF16)
                nc.vector.tensor_copy(hsl[:], tp[:])

                inb = dram.tile([HS, B], BF16)
                nc.scalar.dma_start(out=inb[:], in_=hsl[:])
                outb = dram.tile([NC * HS, B], BF16)
                nc.gpsimd.collective_compute(
                    "AllGather",
                    mybir.AluOpType.bypass,
                    replica_groups=rg,
                    ins=[inb.opt()],
                    outs=[outb.opt()],
                )
                hT = htp.tile([128, KCH, B], BF16)
                nc.sync.dma_start(
                    out=hT[:], in_=outb[:].rearrange("(r p) n -> p r n", p=128)
                )

            nc.scalar.dma_start(out=out_h[t], in_=hnew[:])

        nc.sync.dma_start(out=c_out[:], in_=c_t[:])

    _split_multi_waits(nc)
    return nc


def _host_emb_proj(emb, W_ih, b_ih, b_hh, tgt):
    used, tgt_c = np.unique(tgt, return_inverse=True)
    tgt_c = tgt_c.reshape(tgt.shape)
    bias = (b_ih + b_hh).astype(np.float32)
    ep = emb[used].astype(np.float32) @ W_ih.T.astype(np.float32) + bias
    return ep.astype(np.float32), tgt_c.astype(np.int32)


def _prep_core_inputs(tgt_c, h0, c0, emb_proj_full, W_hh):
    from ml_dtypes import bfloat16

    def slice_rows(M, k):  # rows for core k in [g,i,f,o] order
        return np.concatenate(
            [
                M[2 * H + k * HS : 2 * H + (k + 1) * HS],
                M[0 * H + k * HS : 0 * H + (k + 1) * HS],
                M[1 * H + k * HS : 1 * H + (k + 1) * HS],
                M[3 * H + k * HS : 3 * H + (k + 1) * HS],
            ],
            axis=0,
        )

    h0T_np = (
        np.ascontiguousarray(h0[0].astype(np.float32).T)
        .reshape(KCH, 128, B)
        .transpose(1, 0, 2)
        .astype(bfloat16)
    )
    in_maps = []
    for k in range(NC):
        w_sl = slice_rows(W_hh.astype(np.float32), k)        # [G, H]
        w_rec = (
            np.ascontiguousarray(w_sl.T)
            .reshape(KCH, 128, G)
            .transpose(1, 0, 2)
            .astype(bfloat16)
        )
        ep_sl = np.ascontiguousarray(slice_rows(emb_proj_full.T, k).T)
        in_maps.append(
            {
                "w_rec": np.ascontiguousarray(w_rec),
                "emb_proj": ep_sl.astype(bfloat16),
                "tgt_idx": np.ascontiguousarray(tgt_c),
                "h0T": np.ascontiguousarray(h0T_np),
                "c0_s": np.ascontiguousarray(
                    c0[0].astype(np.float32)[:, k * HS : (k + 1) * HS]
                ),
            }
        )
    return in_maps


_CACHE = {}


def _run(inputs, trace=False):
    tgt = np.asarray(inputs["tgt"])
    h0 = np.asarray(inputs["h0"])
    c0 = np.asarray(inputs["c0"])
    emb = np.asarray(inputs["emb"])
    W_ih = np.asarray(inputs["W_ih"])
    W_hh = np.asarray(inputs["W_hh"])
    b_ih = np.asarray(inputs["b_ih"])
    b_hh = np.asarray(inputs["b_hh"])

    ep, tgt_c = _host_emb_proj(emb, W_ih, b_ih, b_hh, tgt)
    in_maps = _prep_core_inputs(tgt_c, h0, c0, ep, W_hh)

    key = ("nc", ep.shape[0])
    if key not in _CACHE:
        _CACHE[key] = _build_nc(ep.shape[0])
    nc = _CACHE[key]

    res = run_bass_kernel_spmd(nc, in_maps, core_ids=list(range(NC)), trace=trace)

    outs = np.concatenate([res.results[k]["out_h"] for k in range(NC)], axis=2)
    outputs = np.ascontiguousarray(outs.transpose(1, 0, 2)).astype(np.float32)
    c_fin = np.concatenate([res.results[k]["c_out"] for k in range(NC)], axis=1)[None]
    h_fin = np.ascontiguousarray(outputs[:, T - 1, :])[None]
    return (outputs, (h_fin.astype(np.float32), c_fin.astype(np.float32))), res


def kernel(**inputs):
    out, _ = _run(inputs, trace=False)
    return out
